# revision 6
# baseline (speedup 1.0000x reference)
"""MADPSNet MoE-routing kernel for 8 Trainium2 NeuronCores.

The reference computes every expert on the full stacked input and then
gathers one expert per agent.  The routing indices (laac_shallow /
laac_deep) are host-visible numpy values, so we do the routing on the
host: per agent we select the 4 weight matrices of its chosen experts
and run only the selected chain

    x[2048,256] @ W1[256,512] -> relu -> @ W2[512,256] -> relu
                -> @ W3[256,512] -> relu -> @ W4[512,128] (+bias)

One agent per NeuronCore (A == 8 == n_cores), no collectives.

Layout: everything feature-major on chip (features on the 128
partitions, batch on the free dim).  The host pre-packs

    x   [128, 4096]     col = bt*1024 + k*512 + b  (bt-major batch tiles)
    wN  [128, K/128*M]  col = (k*mc + m)*128 + j   (k-chunk-major)
    bias[128, 11]       col j = 128-chunk j of [b1(4) b2(2) b3(4) b4(1)]

All tensors stream as bf16 (accumulate fp32 in PSUM; the 2e-2 rel-err
budget leaves ~5x headroom), which halves HBM traffic and lets
LDWEIGHTS use FWL.  Every DMA is a large contiguous transfer, issued
in compute-need order on the two HWDGE queues (x on sync/SP, weights
on scalar/ACT).  Matmuls accumulate fp32 in PSUM, bias+relu runs split
across ScalarE and VectorE with a fixed engine per destination tile,
and the layers are emitted as a (bt + 2*layer) diagonal wavefront so
the in-order PE queue always has ready work while L1 waits on x DMAs.
A few warm-up matmuls on an uninitialized scratch tile (no deps, so
they issue the moment the PE queue opens) keep the PE busy from kernel
start so the HAM clock un-throttles (1.2 -> 2.4 GHz) before the real
work arrives.  The kernel returns out^T [128, 2048] bf16 per core; the
host transposes back and upcasts.
"""

import os

import numpy as np

import concourse.bass as bass
import concourse.mybir as mybir
from concourse import bacc
from concourse.bass_utils import run_bass_kernel_spmd
from concourse.tile import TileContext

A, B, S = 8, 2048, 256
H1, H2, D1, D2 = 512, 256, 512, 128
P = 128
BT = 512            # batch tile (psum bank: 512 fp32)
NBT = B // BT

_DT_MAP = {
    "f32": mybir.dt.float32,
    "f32r": mybir.dt.float32r,
    "bf16": mybir.dt.bfloat16,
}

# layer: (k_chunks, m_chunks, bias col offset, relu?)
_LAYERS = [
    (S // P, H1 // P, 0, True),    # L1: 256 -> 512
    (H1 // P, H2 // P, 4, True),   # L2: 512 -> 256
    (H2 // P, D1 // P, 6, True),   # L3: 256 -> 512
    (D1 // P, D2 // P, 10, False), # L4: 512 -> 128
]


def _build(dt_name: str, add_bias: bool, warm: int, junk: int) -> bass.Bass:
    dt = _DT_MAP[dt_name]
    f32 = mybir.dt.float32
    nc = bacc.Bacc(None, target_bir_lowering=False, debug=False)

    x_d = nc.dram_tensor("x", [P, (S // P) * B], dt, kind="ExternalInput")
    w_ds = [
        nc.dram_tensor("w1", [P, (S // P) * H1], dt, kind="ExternalInput"),
        nc.dram_tensor("w2", [P, (H1 // P) * H2], dt, kind="ExternalInput"),
        nc.dram_tensor("w3", [P, (H2 // P) * D1], dt, kind="ExternalInput"),
        nc.dram_tensor("w4", [P, (D1 // P) * D2], dt, kind="ExternalInput"),
    ]
    b_d = (
        nc.dram_tensor("bias", [P, 11], f32, kind="ExternalInput")
        if add_bias
        else None
    )
    out_dt = dt if dt == mybir.dt.bfloat16 else f32
    out_d = nc.dram_tensor("out", [D2, B], out_dt, kind="ExternalOutput")

    with TileContext(nc) as tc:
        with (
            tc.tile_pool(name="persist", bufs=1) as pp,
            tc.tile_pool(name="psum", bufs=7, space="PSUM") as psp,
            tc.tile_pool(name="jpsum", bufs=1, space="PSUM") as jpsp,
        ):
            xt = pp.tile([P, (S // P) * B], dt, tag="xt", name="xt")
            wts = [
                pp.tile(
                    [P, w_ds[i].shape[1]], dt, tag=f"w{i}", name=f"w{i}_sb"
                )
                for i in range(4)
            ]
            bti = (
                pp.tile([P, 11], f32, tag="bias", name="bias_sb")
                if add_bias
                else None
            )
            scr = (
                pp.tile([P, 2], f32, tag="scr", name="scr") if add_bias else None
            )
            acts = [
                [
                    pp.tile([P, B], dt, tag=f"a{li}_{i}", name=f"a{li}_{i}")
                    for i in range(n)
                ]
                for li, n in [(1, H1 // P), (2, H2 // P), (3, D1 // P)]
            ]
            acts.append([pp.tile([P, B], out_dt, tag="ot", name="ot")])

            # ---- PE warm-up: matmuls on a zeroed scratch tile dumped in
            # a dedicated PSUM bank, so the HAM clock gate starts ramping
            # (1.2 -> 2.4 GHz) while the first input DMAs are still in
            # flight.  The memset is the tile's required first write; its
            # cross-engine handoff lands right after the TileContext
            # entry barrier, so the first warm-up matmul issues ~0.5us
            # after the PE queue opens.
            wsb = None
            wps = None
            if warm > 0 or junk > 0:
                wdt = f32 if dt == mybir.dt.float32r else dt
                wsb = pp.tile([P, BT], wdt, tag="wsb", name="wsb")
                nc.gpsimd.memset(wsb[:], 0.0)
                wps = jpsp.tile([P, BT], f32, tag="jps", name="wps")
                lhs = wsb[:, 0:P]
                rhs = wsb[:]
                if dt == mybir.dt.float32r:
                    lhs = lhs.bitcast(dt)
                    rhs = rhs.bitcast(dt)
                for _ in range(warm):
                    nc.tensor.matmul(wps[:], lhs, rhs, start=True, stop=True)

            # ---- input DMAs: x per batch-tile on the sync HWDGE queue,
            # weights on the scalar (ACT) HWDGE queue — two parallel
            # descriptor streams, each transfer issued in the order the
            # wavefront consumes it.  x is host-packed bt-major (col =
            # bt*2*BT + k*BT + b) so per-bt transfers are contiguous.
            kx = S // P

            def x_sl(bt, k, nk=1):
                return slice((bt * kx + k) * BT, (bt * kx + k + nk) * BT)

            # The sync (SP) HWDGE ring reaches its first packet ~0.8us
            # sooner than the scalar (ACT) ring, so the two transfers on
            # L1-bt0's critical path -- w1's k0/m01 quarter and x bt0/k0
            # -- lead the sync queue.  Everything else is split to land
            # just ahead of the wavefront's demand.
            nc.sync.dma_start(wts[0][:, 0:256], w_ds[0][:, 0:256])
            sl = x_sl(0, 0)
            nc.sync.dma_start(xt[:, sl], x_d[:, sl])
            sl = x_sl(0, 1)
            nc.sync.dma_start(xt[:, sl], x_d[:, sl])
            sl = x_sl(1, 0, 2)
            nc.sync.dma_start(xt[:, sl], x_d[:, sl])
            sl = x_sl(2, 0, 2)
            nc.sync.dma_start(xt[:, sl], x_d[:, sl])
            sl = x_sl(3, 0, 2)
            nc.sync.dma_start(xt[:, sl], x_d[:, sl])
            # scalar queue: the rest of the weights, first-needed-first
            nc.scalar.dma_start(wts[0][:, 256:512], w_ds[0][:, 256:512])
            nc.scalar.dma_start(wts[0][:, 512:1024], w_ds[0][:, 512:1024])
            nc.scalar.dma_start(wts[1][:], w_ds[1][:])
            nc.scalar.dma_start(wts[2][:], w_ds[2][:])
            nc.scalar.dma_start(wts[3][:], w_ds[3][:])
            if add_bias:
                nc.scalar.dma_start(bti[:], b_d[:])
            if add_bias:
                # advance ACT/DVE engine clocks past the bias DMA so the
                # real post-matmul ops carry a single (PE) wait each — the
                # AC/DVE instruction structs have one wait slot.
                nc.scalar.copy(scr[:, 0:1], bti[:, 0:1])
                nc.vector.tensor_copy(scr[:, 1:2], bti[:, 0:1])

            # ---- the 4-layer chain, emitted as a (bt + 2*layer) diagonal
            # wavefront: the PE's in-order queue then always has ready
            # later-layer work to chew while L1 waits on x DMAs.
            def x_rhs(k, bt):
                return xt[:, (bt * kx + k) * BT : (bt * kx + k + 1) * BT]

            sched = sorted(
                ((bt + 2 * li, -li, bt) for li in range(4) for bt in range(NBT))
            )
            for _, nli, bt in sched:
                li = -nli
                kc, mc, boff, relu = _LAYERS[li]
                wt = wts[li]
                dsts = acts[li]
                srcs = acts[li - 1] if li > 0 else None
                if li == 0:
                    # k-outer for every L1 batch-tile: each k sweep needs
                    # only one x chunk + half of w1 in SBUF, so the
                    # supply-paced phase runs with fine-grained waits
                    pss = [
                        psp.tile([P, BT], f32, tag="ps", name=f"ps_l0_{bt}_{m}")
                        for m in range(mc)
                    ]
                    for k in range(kc):
                        for m in range(mc):
                            nc.tensor.matmul(
                                pss[m][:],
                                wt[:, (k * mc + m) * P : (k * mc + m + 1) * P],
                                x_rhs(k, bt),
                                start=(k == 0),
                                stop=(k == kc - 1),
                            )
                else:
                    pss = None
                if li == 3 and bt == NBT - 1 and not add_bias:
                    # Last batch-tile of the last layer: column-quartered
                    # accumulate -> copy -> DMA chains, so the first
                    # output quarter is in flight while the PE still
                    # works on the later quarters.  This shortens the
                    # post-matmul drain (which otherwise burns the HAM
                    # clock-gate hysteresis before the framework's
                    # semaphore teardown runs).
                    ot = acts[3][0]
                    q = BT // 4
                    o = bt * BT
                    ps = psp.tile([P, BT], f32, tag="ps", name="ps_l3_last")
                    for j in range(4):
                        for k in range(kc):
                            nc.tensor.matmul(
                                ps[:, j * q : (j + 1) * q],
                                wt[:, k * P : (k + 1) * P],
                                srcs[k][:, o + j * q : o + (j + 1) * q],
                                start=(k == 0),
                                stop=(k == kc - 1),
                            )
                        nc.vector.tensor_copy(
                            ot[:, o + j * q : o + (j + 1) * q],
                            ps[:, j * q : (j + 1) * q],
                        )
                        eng = nc.sync if j % 2 == 0 else nc.scalar
                        eng.dma_start(
                            out_d[:, o + j * q : o + (j + 1) * q],
                            ot[:, o + j * q : o + (j + 1) * q],
                        )
                    continue
                for m in range(mc):
                    # fixed engine per dst tile: one writer per tile
                    use_act = (li < 3) and (m < mc // 2 or mc == 1)
                    if pss is not None:
                        ps = pss[m]
                    else:
                        ps = psp.tile([P, BT], f32, tag="ps", name="ps")
                        for k in range(kc):
                            rhs = (
                                x_rhs(k, bt)
                                if li == 0
                                else srcs[k][:, bt * BT : (bt + 1) * BT]
                            )
                            nc.tensor.matmul(
                                ps[:],
                                wt[:, (k * mc + m) * P : (k * mc + m + 1) * P],
                                rhs,
                                start=(k == 0),
                                stop=(k == kc - 1),
                            )
                    dst = dsts[m][:, bt * BT : (bt + 1) * BT]
                    if add_bias:
                        bias_ap = bti[:, boff + m : boff + m + 1]
                        if use_act:
                            func = (
                                mybir.ActivationFunctionType.Relu
                                if relu
                                else mybir.ActivationFunctionType.Identity
                            )
                            nc.scalar.activation(
                                dst, ps[:], func, bias=bias_ap
                            )
                        elif relu:
                            nc.vector.tensor_scalar(
                                dst,
                                ps[:],
                                bias_ap,
                                0.0,
                                mybir.AluOpType.add,
                                mybir.AluOpType.max,
                            )
                        else:
                            nc.vector.tensor_scalar_add(dst, ps[:], bias_ap)
                    elif use_act:
                        func = (
                            mybir.ActivationFunctionType.Relu
                            if relu
                            else mybir.ActivationFunctionType.Copy
                        )
                        nc.scalar.activation(dst, ps[:], func)
                    elif relu:
                        nc.vector.tensor_scalar_max(dst, ps[:], 0.0)
                    elif li == 3 and bt == NBT - 1:
                        # quarter the last copy so the final out-DMA
                        # chunks are small and start early
                        q = BT // 4
                        for j in range(4):
                            nc.vector.tensor_copy(
                                dst[:, j * q : (j + 1) * q],
                                ps[:, j * q : (j + 1) * q],
                            )
                    else:
                        nc.vector.tensor_copy(dst, ps[:])
                if li == 3:
                    ot = acts[3][0]
                    if bt < NBT - 1:
                        eng = nc.sync if bt % 2 == 0 else nc.scalar
                        eng.dma_start(
                            out_d[:, bt * BT : (bt + 1) * BT],
                            ot[:, bt * BT : (bt + 1) * BT],
                        )
                    else:
                        # last tile: quarter across both queues to
                        # shorten the final drain
                        q = BT // 4
                        o = bt * BT
                        for j in range(4):
                            eng = nc.sync if j % 2 == 0 else nc.scalar
                            eng.dma_start(
                                out_d[:, o + j * q : o + (j + 1) * q],
                                ot[:, o + j * q : o + (j + 1) * q],
                            )

            # ---- PE tail-pad: junk matmuls (same scratch operands, no
            # deps beyond program order on the PE queue) issued after the
            # last real matmul.  They keep the PE busy while the final
            # activations/out-DMAs drain, so the HAM clock stays at 8/8
            # through the framework's semaphore-teardown phase instead of
            # dropping to 4/8 (which doubles the teardown's ~3.5us).
            if junk > 0:
                lhs = wsb[:, 0:P]
                rhs = wsb[:]
                if dt == mybir.dt.float32r:
                    lhs = lhs.bitcast(dt)
                    rhs = rhs.bitcast(dt)
                for _ in range(junk):
                    nc.tensor.matmul(wps[:], lhs, rhs, start=True, stop=True)
    nc.compile()
    return nc


_BUILT: dict[tuple, bass.Bass] = {}


def _cfg():
    dt_name = os.environ.get("MADPS_DT", "bf16")
    warm = int(os.environ.get("MADPS_WARM", "3"))
    junk = int(os.environ.get("MADPS_JUNK", "0"))
    return dt_name, warm, junk


def _get_nc(dt_name: str, add_bias: bool, warm: int, junk: int) -> bass.Bass:
    key = (dt_name, add_bias, warm, junk)
    if key not in _BUILT:
        _BUILT[key] = _build(dt_name, add_bias, warm, junk)
    return _BUILT[key]


def _np_dt(dt_name: str):
    if dt_name == "bf16":
        import ml_dtypes

        return ml_dtypes.bfloat16
    return np.float32


def _packw(w: np.ndarray, np_dt) -> np.ndarray:
    """[K, M] -> [128, (K/128)*M], k-chunk-major: col (k*mc + m)*128 + j."""
    k, m = w.shape
    kc = k // P
    return np.ascontiguousarray(
        w.reshape(kc, P, m).transpose(1, 0, 2).reshape(P, -1).astype(np_dt)
    )


def _prepare(inputs, dt_name):
    """Returns (add_bias, in_maps) for run_bass_kernel_spmd."""
    np_dt = _np_dt(dt_name)

    x = np.asarray(inputs["inputs"], dtype=np.float32)
    sel_s = np.asarray(inputs["laac_shallow"]).reshape(-1).astype(np.int64)
    sel_d = np.asarray(inputs["laac_deep"]).reshape(-1).astype(np.int64)
    Ws1 = np.asarray(inputs["Ws1"], dtype=np.float32)
    Ws2 = np.asarray(inputs["Ws2"], dtype=np.float32)
    Wd1 = np.asarray(inputs["Wd1"], dtype=np.float32)
    Wd2 = np.asarray(inputs["Wd2"], dtype=np.float32)
    bs1 = np.asarray(inputs["bs1"], dtype=np.float32)
    bs2 = np.asarray(inputs["bs2"], dtype=np.float32)
    bd1 = np.asarray(inputs["bd1"], dtype=np.float32)
    bd2 = np.asarray(inputs["bd2"], dtype=np.float32)

    add_bias = any(
        float(np.abs(b).max()) != 0.0 for b in (bs1, bs2, bd1, bd2)
    )

    in_maps = []
    for a in range(A):
        es, ed = int(sel_s[a]), int(sel_d[a])
        # bt-major packing: col = bt*(S//P)*BT + k*BT + b
        xp = np.ascontiguousarray(
            x[a]
            .reshape(NBT, BT, S // P, P)
            .transpose(3, 0, 2, 1)
            .reshape(P, -1)
            .astype(np_dt)
        )
        m = {
            "x": xp,
            "w1": _packw(Ws1[es], np_dt),
            "w2": _packw(Ws2[es], np_dt),
            "w3": _packw(Wd1[ed], np_dt),
            "w4": _packw(Wd2[ed], np_dt),
        }
        if add_bias:
            bias_cols = np.concatenate([bs1[es], bs2[es], bd1[ed], bd2[ed]])
            m["bias"] = np.ascontiguousarray(
                bias_cols.reshape(11, P).T, dtype=np.float32
            )
        in_maps.append(m)
    return add_bias, in_maps


def kernel(**inputs) -> np.ndarray:
    dt_name, warm, junk = _cfg()
    add_bias, in_maps = _prepare(inputs, dt_name)
    nc = _get_nc(dt_name, add_bias, warm, junk)
    res = run_bass_kernel_spmd(nc, in_maps, list(range(A)))
    out = np.stack(
        [np.asarray(res.results[a]["out"]).astype(np.float32).T for a in range(A)]
    )
    return np.ascontiguousarray(out)


# revision 9
# speedup vs baseline: 1.0470x; 1.0470x over previous
"""MADPSNet MoE-routing kernel for 8 Trainium2 NeuronCores.

The reference computes every expert on the full stacked input and then
gathers one expert per agent.  The routing indices (laac_shallow /
laac_deep) are host-visible numpy values, so we do the routing on the
host: per agent we select the 4 weight matrices of its chosen experts
and run only the selected chain

    x[2048,256] @ W1[256,512] -> relu -> @ W2[512,256] -> relu
                -> @ W3[256,512] -> relu -> @ W4[512,128] (+bias)

One agent per NeuronCore (A == 8 == n_cores), no collectives.

Layout: everything feature-major on chip (features on the 128
partitions, batch on the free dim).  The host pre-packs

    x   [128, 4096]     col = bt*1024 + k*512 + b  (bt-major batch tiles)
    wN  [128, K/128*M]  col = (k*mc + m)*128 + j   (k-chunk-major)
    bias[128, 11]       col j = 128-chunk j of [b1(4) b2(2) b3(4) b4(1)]

All tensors stream as bf16 (accumulate fp32 in PSUM; the 2e-2 rel-err
budget leaves ~5x headroom), which halves HBM traffic and lets
LDWEIGHTS use FWL.  Every DMA is a large contiguous transfer, issued
in compute-need order on the two HWDGE queues (x on sync/SP, weights
on scalar/ACT).  Matmuls accumulate fp32 in PSUM, bias+relu runs split
across ScalarE and VectorE with a fixed engine per destination tile,
and the layers are emitted as a (bt + 2*layer) diagonal wavefront so
the in-order PE queue always has ready work while L1 waits on x DMAs.
A few warm-up matmuls on an uninitialized scratch tile (no deps, so
they issue the moment the PE queue opens) keep the PE busy from kernel
start so the HAM clock un-throttles (1.2 -> 2.4 GHz) before the real
work arrives.  The kernel returns out^T [128, 2048] bf16 per core; the
host transposes back and upcasts.
"""

import os

import numpy as np

import concourse.bass as bass
import concourse.mybir as mybir
from concourse import bacc
from concourse.bass_utils import run_bass_kernel_spmd
from concourse.tile import TileContext

A, B, S = 8, 2048, 256
H1, H2, D1, D2 = 512, 256, 512, 128
P = 128
BT = 512            # batch tile (psum bank: 512 fp32)
NBT = B // BT

_DT_MAP = {
    "f32": mybir.dt.float32,
    "f32r": mybir.dt.float32r,
    "bf16": mybir.dt.bfloat16,
}

# layer: (k_chunks, m_chunks, bias col offset, relu?)
_LAYERS = [
    (S // P, H1 // P, 0, True),    # L1: 256 -> 512
    (H1 // P, H2 // P, 4, True),   # L2: 512 -> 256
    (H2 // P, D1 // P, 6, True),   # L3: 256 -> 512
    (D1 // P, D2 // P, 10, False), # L4: 512 -> 128
]


def _build(dt_name: str, add_bias: bool, warm: int, junk: int) -> bass.Bass:
    dt = _DT_MAP[dt_name]
    f32 = mybir.dt.float32
    nc = bacc.Bacc(None, target_bir_lowering=False, debug=False)

    x_d = nc.dram_tensor("x", [P, (S // P) * B], dt, kind="ExternalInput")
    w_ds = [
        nc.dram_tensor("w1", [P, (S // P) * H1], dt, kind="ExternalInput"),
        nc.dram_tensor("w2", [P, (H1 // P) * H2], dt, kind="ExternalInput"),
        nc.dram_tensor("w3", [P, (H2 // P) * D1], dt, kind="ExternalInput"),
        nc.dram_tensor("w4", [P, (D1 // P) * D2], dt, kind="ExternalInput"),
    ]
    b_d = (
        nc.dram_tensor("bias", [P, 11], f32, kind="ExternalInput")
        if add_bias
        else None
    )
    out_dt = dt if dt == mybir.dt.bfloat16 else f32
    out_d = nc.dram_tensor("out", [D2, B], out_dt, kind="ExternalOutput")

    with TileContext(nc) as tc:
        with (
            tc.tile_pool(name="persist", bufs=1) as pp,
            tc.tile_pool(name="psum", bufs=7, space="PSUM") as psp,
            tc.tile_pool(name="jpsum", bufs=1, space="PSUM") as jpsp,
        ):
            xt = pp.tile([P, (S // P) * B], dt, tag="xt", name="xt")
            wts = [
                pp.tile(
                    [P, w_ds[i].shape[1]], dt, tag=f"w{i}", name=f"w{i}_sb"
                )
                for i in range(4)
            ]
            bti = (
                pp.tile([P, 11], f32, tag="bias", name="bias_sb")
                if add_bias
                else None
            )
            scr = (
                pp.tile([P, 2], f32, tag="scr", name="scr") if add_bias else None
            )
            acts = [
                [
                    pp.tile([P, B], dt, tag=f"a{li}_{i}", name=f"a{li}_{i}")
                    for i in range(n)
                ]
                for li, n in [(1, H1 // P), (2, H2 // P), (3, D1 // P)]
            ]
            acts.append([pp.tile([P, B], out_dt, tag="ot", name="ot")])

            # ---- PE warm-up: matmuls on a zeroed scratch tile dumped in
            # a dedicated PSUM bank, so the HAM clock gate starts ramping
            # (1.2 -> 2.4 GHz) while the first input DMAs are still in
            # flight.  The memset is the tile's required first write; its
            # cross-engine handoff lands right after the TileContext
            # entry barrier, so the first warm-up matmul issues ~0.5us
            # after the PE queue opens.
            wsb = None
            wps = None
            if warm > 0 or junk > 0:
                wdt = f32 if dt == mybir.dt.float32r else dt
                wsb = pp.tile([P, BT], wdt, tag="wsb", name="wsb")
                nc.gpsimd.memset(wsb[:], 0.0)
                wps = jpsp.tile([P, BT], f32, tag="jps", name="wps")
                lhs = wsb[:, 0:P]
                rhs = wsb[:]
                if dt == mybir.dt.float32r:
                    lhs = lhs.bitcast(dt)
                    rhs = rhs.bitcast(dt)
                for _ in range(warm):
                    nc.tensor.matmul(wps[:], lhs, rhs, start=True, stop=True)

            # ---- input DMAs: x per batch-tile on the sync HWDGE queue,
            # weights on the scalar (ACT) HWDGE queue — two parallel
            # descriptor streams, each transfer issued in the order the
            # wavefront consumes it.  x is host-packed bt-major (col =
            # bt*2*BT + k*BT + b) so per-bt transfers are contiguous.
            kx = S // P

            def x_sl(bt, k, nk=1):
                return slice((bt * kx + k) * BT, (bt * kx + k + nk) * BT)

            # scalar queue: weights, first-needed-first
            nc.scalar.dma_start(wts[0][:, 0:512], w_ds[0][:, 0:512])
            nc.scalar.dma_start(wts[0][:, 512:1024], w_ds[0][:, 512:1024])
            nc.scalar.dma_start(wts[1][:], w_ds[1][:])
            nc.scalar.dma_start(wts[3][:], w_ds[3][:])
            if add_bias:
                nc.scalar.dma_start(bti[:], b_d[:])
            # sync queue: x batch-tiles in wavefront order, w3 slotted in
            # after x bt2 (w3 is first needed at wavefront key 4).
            sl = x_sl(0, 0)
            nc.sync.dma_start(xt[:, sl], x_d[:, sl])
            sl = x_sl(0, 1)
            nc.sync.dma_start(xt[:, sl], x_d[:, sl])
            sl = x_sl(1, 0, 2)
            nc.sync.dma_start(xt[:, sl], x_d[:, sl])
            sl = x_sl(2, 0, 2)
            nc.sync.dma_start(xt[:, sl], x_d[:, sl])
            nc.sync.dma_start(wts[2][:], w_ds[2][:])
            sl = x_sl(3, 0, 2)
            nc.sync.dma_start(xt[:, sl], x_d[:, sl])
            if add_bias:
                # advance ACT/DVE engine clocks past the bias DMA so the
                # real post-matmul ops carry a single (PE) wait each — the
                # AC/DVE instruction structs have one wait slot.
                nc.scalar.copy(scr[:, 0:1], bti[:, 0:1])
                nc.vector.tensor_copy(scr[:, 1:2], bti[:, 0:1])

            # ---- the 4-layer chain, emitted as a (bt + 2*layer) diagonal
            # wavefront: the PE's in-order queue then always has ready
            # later-layer work to chew while L1 waits on x DMAs.
            def x_rhs(k, bt):
                return xt[:, (bt * kx + k) * BT : (bt * kx + k + 1) * BT]

            sched = sorted(
                ((bt + 2 * li, -li, bt) for li in range(4) for bt in range(NBT))
            )
            for _, nli, bt in sched:
                li = -nli
                kc, mc, boff, relu = _LAYERS[li]
                wt = wts[li]
                dsts = acts[li]
                srcs = acts[li - 1] if li > 0 else None
                if li == 0:
                    # k-outer for every L1 batch-tile: each k sweep needs
                    # only one x chunk + half of w1 in SBUF, so the
                    # supply-paced phase runs with fine-grained waits
                    pss = [
                        psp.tile([P, BT], f32, tag="ps", name=f"ps_l0_{bt}_{m}")
                        for m in range(mc)
                    ]
                    for k in range(kc):
                        for m in range(mc):
                            nc.tensor.matmul(
                                pss[m][:],
                                wt[:, (k * mc + m) * P : (k * mc + m + 1) * P],
                                x_rhs(k, bt),
                                start=(k == 0),
                                stop=(k == kc - 1),
                            )
                else:
                    pss = None
                if li == 3 and bt == NBT - 1 and not add_bias:
                    # Last batch-tile of the last layer: column halves in
                    # two separate PSUM tiles (a shared tile would WAR-
                    # serialize half 1's first matmul behind half 0's
                    # PSUM->SBUF cast), each half cast and DMA'd on its
                    # own queue the moment it completes.  This shortens
                    # the post-matmul drain, which otherwise burns the
                    # HAM clock-gate hysteresis before the framework's
                    # semaphore teardown runs.
                    ot = acts[3][0]
                    h = BT // 2
                    o = bt * BT
                    for j in range(2):
                        ps = psp.tile([P, h], f32, tag="ps", name=f"ps_l3h{j}")
                        for k in range(kc):
                            nc.tensor.matmul(
                                ps[:],
                                wt[:, k * P : (k + 1) * P],
                                srcs[k][:, o + j * h : o + (j + 1) * h],
                                start=(k == 0),
                                stop=(k == kc - 1),
                            )
                        nc.vector.tensor_copy(
                            ot[:, o + j * h : o + (j + 1) * h], ps[:]
                        )
                        eng = nc.sync if j == 0 else nc.scalar
                        eng.dma_start(
                            out_d[:, o + j * h : o + (j + 1) * h],
                            ot[:, o + j * h : o + (j + 1) * h],
                        )
                    continue
                for m in range(mc):
                    # fixed engine per dst tile: one writer per tile
                    use_act = (li < 3) and (m < mc // 2 or mc == 1)
                    if pss is not None:
                        ps = pss[m]
                    else:
                        ps = psp.tile([P, BT], f32, tag="ps", name="ps")
                        for k in range(kc):
                            rhs = (
                                x_rhs(k, bt)
                                if li == 0
                                else srcs[k][:, bt * BT : (bt + 1) * BT]
                            )
                            nc.tensor.matmul(
                                ps[:],
                                wt[:, (k * mc + m) * P : (k * mc + m + 1) * P],
                                rhs,
                                start=(k == 0),
                                stop=(k == kc - 1),
                            )
                    dst = dsts[m][:, bt * BT : (bt + 1) * BT]
                    if add_bias:
                        bias_ap = bti[:, boff + m : boff + m + 1]
                        if use_act:
                            func = (
                                mybir.ActivationFunctionType.Relu
                                if relu
                                else mybir.ActivationFunctionType.Identity
                            )
                            nc.scalar.activation(
                                dst, ps[:], func, bias=bias_ap
                            )
                        elif relu:
                            nc.vector.tensor_scalar(
                                dst,
                                ps[:],
                                bias_ap,
                                0.0,
                                mybir.AluOpType.add,
                                mybir.AluOpType.max,
                            )
                        else:
                            nc.vector.tensor_scalar_add(dst, ps[:], bias_ap)
                    elif use_act:
                        func = (
                            mybir.ActivationFunctionType.Relu
                            if relu
                            else mybir.ActivationFunctionType.Copy
                        )
                        nc.scalar.activation(dst, ps[:], func)
                    elif relu:
                        nc.vector.tensor_scalar_max(dst, ps[:], 0.0)
                    elif li == 3 and bt == NBT - 1:
                        # quarter the last copy so the final out-DMA
                        # chunks are small and start early
                        q = BT // 4
                        for j in range(4):
                            nc.vector.tensor_copy(
                                dst[:, j * q : (j + 1) * q],
                                ps[:, j * q : (j + 1) * q],
                            )
                    else:
                        nc.vector.tensor_copy(dst, ps[:])
                if li == 3:
                    ot = acts[3][0]
                    if bt < NBT - 1:
                        eng = nc.sync if bt % 2 == 0 else nc.scalar
                        eng.dma_start(
                            out_d[:, bt * BT : (bt + 1) * BT],
                            ot[:, bt * BT : (bt + 1) * BT],
                        )
                    else:
                        # last tile: quarter across both queues to
                        # shorten the final drain
                        q = BT // 4
                        o = bt * BT
                        for j in range(4):
                            eng = nc.sync if j % 2 == 0 else nc.scalar
                            eng.dma_start(
                                out_d[:, o + j * q : o + (j + 1) * q],
                                ot[:, o + j * q : o + (j + 1) * q],
                            )

            # ---- PE tail-pad: junk matmuls (same scratch operands, no
            # deps beyond program order on the PE queue) issued after the
            # last real matmul.  They keep the PE busy while the final
            # activations/out-DMAs drain, so the HAM clock stays at 8/8
            # through the framework's semaphore-teardown phase instead of
            # dropping to 4/8 (which doubles the teardown's ~3.5us).
            if junk > 0:
                lhs = wsb[:, 0:P]
                rhs = wsb[:]
                if dt == mybir.dt.float32r:
                    lhs = lhs.bitcast(dt)
                    rhs = rhs.bitcast(dt)
                for _ in range(junk):
                    nc.tensor.matmul(wps[:], lhs, rhs, start=True, stop=True)
    nc.compile()
    return nc


_BUILT: dict[tuple, bass.Bass] = {}


def _cfg():
    dt_name = os.environ.get("MADPS_DT", "bf16")
    warm = int(os.environ.get("MADPS_WARM", "4"))
    junk = int(os.environ.get("MADPS_JUNK", "0"))
    return dt_name, warm, junk


def _get_nc(dt_name: str, add_bias: bool, warm: int, junk: int) -> bass.Bass:
    key = (dt_name, add_bias, warm, junk)
    if key not in _BUILT:
        _BUILT[key] = _build(dt_name, add_bias, warm, junk)
    return _BUILT[key]


def _np_dt(dt_name: str):
    if dt_name == "bf16":
        import ml_dtypes

        return ml_dtypes.bfloat16
    return np.float32


def _packw(w: np.ndarray, np_dt) -> np.ndarray:
    """[K, M] -> [128, (K/128)*M], k-chunk-major: col (k*mc + m)*128 + j."""
    k, m = w.shape
    kc = k // P
    return np.ascontiguousarray(
        w.reshape(kc, P, m).transpose(1, 0, 2).reshape(P, -1).astype(np_dt)
    )


def _prepare(inputs, dt_name):
    """Returns (add_bias, in_maps) for run_bass_kernel_spmd."""
    np_dt = _np_dt(dt_name)

    x = np.asarray(inputs["inputs"], dtype=np.float32)
    sel_s = np.asarray(inputs["laac_shallow"]).reshape(-1).astype(np.int64)
    sel_d = np.asarray(inputs["laac_deep"]).reshape(-1).astype(np.int64)
    Ws1 = np.asarray(inputs["Ws1"], dtype=np.float32)
    Ws2 = np.asarray(inputs["Ws2"], dtype=np.float32)
    Wd1 = np.asarray(inputs["Wd1"], dtype=np.float32)
    Wd2 = np.asarray(inputs["Wd2"], dtype=np.float32)
    bs1 = np.asarray(inputs["bs1"], dtype=np.float32)
    bs2 = np.asarray(inputs["bs2"], dtype=np.float32)
    bd1 = np.asarray(inputs["bd1"], dtype=np.float32)
    bd2 = np.asarray(inputs["bd2"], dtype=np.float32)

    add_bias = any(
        float(np.abs(b).max()) != 0.0 for b in (bs1, bs2, bd1, bd2)
    )

    in_maps = []
    for a in range(A):
        es, ed = int(sel_s[a]), int(sel_d[a])
        # bt-major packing: col = bt*(S//P)*BT + k*BT + b
        xp = np.ascontiguousarray(
            x[a]
            .reshape(NBT, BT, S // P, P)
            .transpose(3, 0, 2, 1)
            .reshape(P, -1)
            .astype(np_dt)
        )
        m = {
            "x": xp,
            "w1": _packw(Ws1[es], np_dt),
            "w2": _packw(Ws2[es], np_dt),
            "w3": _packw(Wd1[ed], np_dt),
            "w4": _packw(Wd2[ed], np_dt),
        }
        if add_bias:
            bias_cols = np.concatenate([bs1[es], bs2[es], bd1[ed], bd2[ed]])
            m["bias"] = np.ascontiguousarray(
                bias_cols.reshape(11, P).T, dtype=np.float32
            )
        in_maps.append(m)
    return add_bias, in_maps


def kernel(**inputs) -> np.ndarray:
    dt_name, warm, junk = _cfg()
    add_bias, in_maps = _prepare(inputs, dt_name)
    nc = _get_nc(dt_name, add_bias, warm, junk)
    res = run_bass_kernel_spmd(nc, in_maps, list(range(A)))
    out = np.stack(
        [np.asarray(res.results[a]["out"]).astype(np.float32).T for a in range(A)]
    )
    return np.ascontiguousarray(out)


# revision 10
# speedup vs baseline: 1.0817x; 1.0332x over previous
"""MADPSNet MoE-routing kernel for 8 Trainium2 NeuronCores.

The reference computes every expert on the full stacked input and then
gathers one expert per agent.  The routing indices (laac_shallow /
laac_deep) are host-visible numpy values, so we do the routing on the
host: per agent we select the 4 weight matrices of its chosen experts
and run only the selected chain

    x[2048,256] @ W1[256,512] -> relu -> @ W2[512,256] -> relu
                -> @ W3[256,512] -> relu -> @ W4[512,128] (+bias)

One agent per NeuronCore (A == 8 == n_cores), no collectives.

Layout: everything feature-major on chip (features on the 128
partitions, batch on the free dim).  The host pre-packs

    x   [128, 4096]     col = bt*1024 + k*512 + b  (bt-major batch tiles)
    wN  [128, K/128*M]  col = (k*mc + m)*128 + j   (k-chunk-major)
    bias[128, 11]       col j = 128-chunk j of [b1(4) b2(2) b3(4) b4(1)]

All tensors stream as bf16 (accumulate fp32 in PSUM; the 2e-2 rel-err
budget leaves ~5x headroom), which halves HBM traffic and lets
LDWEIGHTS use FWL.  Every DMA is a large contiguous transfer, issued
in compute-need order on the two HWDGE queues (x on sync/SP, weights
on scalar/ACT).  Matmuls accumulate fp32 in PSUM, bias+relu runs split
across ScalarE and VectorE with a fixed engine per destination tile,
and the layers are emitted as a (bt + 2*layer) diagonal wavefront so
the in-order PE queue always has ready work while L1 waits on x DMAs.
A few warm-up matmuls on an uninitialized scratch tile (no deps, so
they issue the moment the PE queue opens) keep the PE busy from kernel
start so the HAM clock un-throttles (1.2 -> 2.4 GHz) before the real
work arrives.  The kernel returns out^T [128, 2048] bf16 per core; the
host transposes back and upcasts.
"""

import os

import numpy as np

import concourse.bass as bass
import concourse.mybir as mybir
from concourse import bacc
from concourse.bass_utils import run_bass_kernel_spmd
from concourse.tile import TileContext

A, B, S = 8, 2048, 256
H1, H2, D1, D2 = 512, 256, 512, 128
P = 128
BT = 512            # batch tile (psum bank: 512 fp32)
NBT = B // BT

_DT_MAP = {
    "f32": mybir.dt.float32,
    "f32r": mybir.dt.float32r,
    "bf16": mybir.dt.bfloat16,
}

# layer: (k_chunks, m_chunks, bias col offset, relu?)
_LAYERS = [
    (S // P, H1 // P, 0, True),    # L1: 256 -> 512
    (H1 // P, H2 // P, 4, True),   # L2: 512 -> 256
    (H2 // P, D1 // P, 6, True),   # L3: 256 -> 512
    (D1 // P, D2 // P, 10, False), # L4: 512 -> 128
]


def _build(dt_name: str, add_bias: bool, warm: int, junk: int) -> bass.Bass:
    dt = _DT_MAP[dt_name]
    f32 = mybir.dt.float32
    nc = bacc.Bacc(None, target_bir_lowering=False, debug=False)

    x_d = nc.dram_tensor("x", [P, (S // P) * B], dt, kind="ExternalInput")
    w_ds = [
        nc.dram_tensor("w1", [P, (S // P) * H1], dt, kind="ExternalInput"),
        nc.dram_tensor("w2", [P, (H1 // P) * H2], dt, kind="ExternalInput"),
        nc.dram_tensor("w3", [P, (H2 // P) * D1], dt, kind="ExternalInput"),
        nc.dram_tensor("w4", [P, (D1 // P) * D2], dt, kind="ExternalInput"),
    ]
    b_d = (
        nc.dram_tensor("bias", [P, 11], f32, kind="ExternalInput")
        if add_bias
        else None
    )
    out_dt = dt if dt == mybir.dt.bfloat16 else f32
    out_d = nc.dram_tensor("out", [D2, B], out_dt, kind="ExternalOutput")

    with TileContext(nc) as tc:
        with (
            tc.tile_pool(name="persist", bufs=1) as pp,
            tc.tile_pool(name="psum", bufs=7, space="PSUM") as psp,
            tc.tile_pool(name="jpsum", bufs=1, space="PSUM") as jpsp,
        ):
            xt = pp.tile([P, (S // P) * B], dt, tag="xt", name="xt")
            wts = [
                pp.tile(
                    [P, w_ds[i].shape[1]], dt, tag=f"w{i}", name=f"w{i}_sb"
                )
                for i in range(4)
            ]
            bti = (
                pp.tile([P, 11], f32, tag="bias", name="bias_sb")
                if add_bias
                else None
            )
            scr = (
                pp.tile([P, 2], f32, tag="scr", name="scr") if add_bias else None
            )
            acts = [
                [
                    pp.tile([P, B], dt, tag=f"a{li}_{i}", name=f"a{li}_{i}")
                    for i in range(n)
                ]
                for li, n in [(1, H1 // P), (2, H2 // P), (3, D1 // P)]
            ]
            acts.append([pp.tile([P, B], out_dt, tag="ot", name="ot")])

            # ---- PE warm-up: matmuls on a zeroed scratch tile dumped in
            # a dedicated PSUM bank, so the HAM clock gate starts ramping
            # (1.2 -> 2.4 GHz) while the first input DMAs are still in
            # flight.  The memset is the tile's required first write; its
            # cross-engine handoff lands right after the TileContext
            # entry barrier, so the first warm-up matmul issues ~0.5us
            # after the PE queue opens.
            wsb = None
            wps = None
            if warm > 0 or junk > 0:
                wdt = f32 if dt == mybir.dt.float32r else dt
                wsb = pp.tile([P, BT], wdt, tag="wsb", name="wsb")
                nc.gpsimd.memset(wsb[:], 0.0)
                wps = jpsp.tile([P, BT], f32, tag="jps", name="wps")
                lhs = wsb[:, 0:P]
                rhs = wsb[:]
                if dt == mybir.dt.float32r:
                    lhs = lhs.bitcast(dt)
                    rhs = rhs.bitcast(dt)
                for _ in range(warm):
                    nc.tensor.matmul(wps[:], lhs, rhs, start=True, stop=True)

            # ---- input DMAs: x per batch-tile on the sync HWDGE queue,
            # weights on the scalar (ACT) HWDGE queue — two parallel
            # descriptor streams, each transfer issued in the order the
            # wavefront consumes it.  x is host-packed bt-major (col =
            # bt*2*BT + k*BT + b) so per-bt transfers are contiguous.
            kx = S // P

            def x_sl(bt, k, nk=1):
                return slice((bt * kx + k) * BT, (bt * kx + k + nk) * BT)

            # scalar queue: weights, first-needed-first
            nc.scalar.dma_start(wts[0][:, 0:512], w_ds[0][:, 0:512])
            nc.scalar.dma_start(wts[0][:, 512:1024], w_ds[0][:, 512:1024])
            nc.scalar.dma_start(wts[1][:], w_ds[1][:])
            nc.scalar.dma_start(wts[3][:], w_ds[3][:])
            if add_bias:
                nc.scalar.dma_start(bti[:], b_d[:])
            # sync queue: x batch-tiles in wavefront order, w3 slotted in
            # after x bt2 (w3 is first needed at wavefront key 4).
            sl = x_sl(0, 0)
            nc.sync.dma_start(xt[:, sl], x_d[:, sl])
            sl = x_sl(0, 1)
            nc.sync.dma_start(xt[:, sl], x_d[:, sl])
            sl = x_sl(1, 0, 2)
            nc.sync.dma_start(xt[:, sl], x_d[:, sl])
            sl = x_sl(2, 0, 2)
            nc.sync.dma_start(xt[:, sl], x_d[:, sl])
            sl = x_sl(3, 0, 2)
            nc.sync.dma_start(xt[:, sl], x_d[:, sl])
            nc.sync.dma_start(wts[2][:], w_ds[2][:])
            if add_bias:
                # advance ACT/DVE engine clocks past the bias DMA so the
                # real post-matmul ops carry a single (PE) wait each — the
                # AC/DVE instruction structs have one wait slot.
                nc.scalar.copy(scr[:, 0:1], bti[:, 0:1])
                nc.vector.tensor_copy(scr[:, 1:2], bti[:, 0:1])

            # ---- the 4-layer chain, emitted as a (bt + 2*layer) diagonal
            # wavefront: the PE's in-order queue then always has ready
            # later-layer work to chew while L1 waits on x DMAs.
            def x_rhs(k, bt):
                return xt[:, (bt * kx + k) * BT : (bt * kx + k + 1) * BT]

            sched = sorted(
                ((bt + 2 * li, -li, bt) for li in range(4) for bt in range(NBT))
            )
            for _, nli, bt in sched:
                li = -nli
                kc, mc, boff, relu = _LAYERS[li]
                wt = wts[li]
                dsts = acts[li]
                srcs = acts[li - 1] if li > 0 else None
                if li == 0:
                    # k-outer for every L1 batch-tile: each k sweep needs
                    # only one x chunk + half of w1 in SBUF, so the
                    # supply-paced phase runs with fine-grained waits
                    pss = [
                        psp.tile([P, BT], f32, tag="ps", name=f"ps_l0_{bt}_{m}")
                        for m in range(mc)
                    ]
                    for k in range(kc):
                        for m in range(mc):
                            nc.tensor.matmul(
                                pss[m][:],
                                wt[:, (k * mc + m) * P : (k * mc + m + 1) * P],
                                x_rhs(k, bt),
                                start=(k == 0),
                                stop=(k == kc - 1),
                            )
                else:
                    pss = None
                if li == 3 and bt == NBT - 1 and not add_bias:
                    # Last batch-tile of the last layer: column halves in
                    # two separate PSUM tiles (a shared tile would WAR-
                    # serialize half 1's first matmul behind half 0's
                    # PSUM->SBUF cast), each half cast and DMA'd on its
                    # own queue the moment it completes.  This shortens
                    # the post-matmul drain, which otherwise burns the
                    # HAM clock-gate hysteresis before the framework's
                    # semaphore teardown runs.
                    ot = acts[3][0]
                    h = BT // 2
                    o = bt * BT
                    for j in range(2):
                        ps = psp.tile([P, h], f32, tag="ps", name=f"ps_l3h{j}")
                        for k in range(kc):
                            nc.tensor.matmul(
                                ps[:],
                                wt[:, k * P : (k + 1) * P],
                                srcs[k][:, o + j * h : o + (j + 1) * h],
                                start=(k == 0),
                                stop=(k == kc - 1),
                            )
                        nc.vector.tensor_copy(
                            ot[:, o + j * h : o + (j + 1) * h], ps[:]
                        )
                        eng = nc.sync if j == 0 else nc.scalar
                        eng.dma_start(
                            out_d[:, o + j * h : o + (j + 1) * h],
                            ot[:, o + j * h : o + (j + 1) * h],
                        )
                    continue
                for m in range(mc):
                    # fixed engine per dst tile: one writer per tile
                    use_act = (li < 3) and (m < mc // 2 or mc == 1)
                    if pss is not None:
                        ps = pss[m]
                    else:
                        ps = psp.tile([P, BT], f32, tag="ps", name="ps")
                        for k in range(kc):
                            rhs = (
                                x_rhs(k, bt)
                                if li == 0
                                else srcs[k][:, bt * BT : (bt + 1) * BT]
                            )
                            nc.tensor.matmul(
                                ps[:],
                                wt[:, (k * mc + m) * P : (k * mc + m + 1) * P],
                                rhs,
                                start=(k == 0),
                                stop=(k == kc - 1),
                            )
                    dst = dsts[m][:, bt * BT : (bt + 1) * BT]
                    if add_bias:
                        bias_ap = bti[:, boff + m : boff + m + 1]
                        if use_act:
                            func = (
                                mybir.ActivationFunctionType.Relu
                                if relu
                                else mybir.ActivationFunctionType.Identity
                            )
                            nc.scalar.activation(
                                dst, ps[:], func, bias=bias_ap
                            )
                        elif relu:
                            nc.vector.tensor_scalar(
                                dst,
                                ps[:],
                                bias_ap,
                                0.0,
                                mybir.AluOpType.add,
                                mybir.AluOpType.max,
                            )
                        else:
                            nc.vector.tensor_scalar_add(dst, ps[:], bias_ap)
                    elif use_act:
                        func = (
                            mybir.ActivationFunctionType.Relu
                            if relu
                            else mybir.ActivationFunctionType.Copy
                        )
                        nc.scalar.activation(dst, ps[:], func)
                    elif relu:
                        nc.vector.tensor_scalar_max(dst, ps[:], 0.0)
                    elif li == 3 and bt == NBT - 1:
                        # quarter the last copy so the final out-DMA
                        # chunks are small and start early
                        q = BT // 4
                        for j in range(4):
                            nc.vector.tensor_copy(
                                dst[:, j * q : (j + 1) * q],
                                ps[:, j * q : (j + 1) * q],
                            )
                    else:
                        nc.vector.tensor_copy(dst, ps[:])
                if li == 3:
                    ot = acts[3][0]
                    if bt < NBT - 1:
                        eng = nc.sync if bt % 2 == 0 else nc.scalar
                        eng.dma_start(
                            out_d[:, bt * BT : (bt + 1) * BT],
                            ot[:, bt * BT : (bt + 1) * BT],
                        )
                    else:
                        # last tile: quarter across both queues to
                        # shorten the final drain
                        q = BT // 4
                        o = bt * BT
                        for j in range(4):
                            eng = nc.sync if j % 2 == 0 else nc.scalar
                            eng.dma_start(
                                out_d[:, o + j * q : o + (j + 1) * q],
                                ot[:, o + j * q : o + (j + 1) * q],
                            )

            # ---- PE tail-pad: junk matmuls (same scratch operands, no
            # deps beyond program order on the PE queue) issued after the
            # last real matmul.  They keep the PE busy while the final
            # activations/out-DMAs drain, so the HAM clock stays at 8/8
            # through the framework's semaphore-teardown phase instead of
            # dropping to 4/8 (which doubles the teardown's ~3.5us).
            if junk > 0:
                lhs = wsb[:, 0:P]
                rhs = wsb[:]
                if dt == mybir.dt.float32r:
                    lhs = lhs.bitcast(dt)
                    rhs = rhs.bitcast(dt)
                for _ in range(junk):
                    nc.tensor.matmul(wps[:], lhs, rhs, start=True, stop=True)
    nc.compile()
    return nc


_BUILT: dict[tuple, bass.Bass] = {}


def _cfg():
    dt_name = os.environ.get("MADPS_DT", "bf16")
    warm = int(os.environ.get("MADPS_WARM", "4"))
    junk = int(os.environ.get("MADPS_JUNK", "0"))
    return dt_name, warm, junk


def _get_nc(dt_name: str, add_bias: bool, warm: int, junk: int) -> bass.Bass:
    key = (dt_name, add_bias, warm, junk)
    if key not in _BUILT:
        _BUILT[key] = _build(dt_name, add_bias, warm, junk)
    return _BUILT[key]


def _np_dt(dt_name: str):
    if dt_name == "bf16":
        import ml_dtypes

        return ml_dtypes.bfloat16
    return np.float32


def _packw(w: np.ndarray, np_dt) -> np.ndarray:
    """[K, M] -> [128, (K/128)*M], k-chunk-major: col (k*mc + m)*128 + j."""
    k, m = w.shape
    kc = k // P
    return np.ascontiguousarray(
        w.reshape(kc, P, m).transpose(1, 0, 2).reshape(P, -1).astype(np_dt)
    )


def _prepare(inputs, dt_name):
    """Returns (add_bias, in_maps) for run_bass_kernel_spmd."""
    np_dt = _np_dt(dt_name)

    x = np.asarray(inputs["inputs"], dtype=np.float32)
    sel_s = np.asarray(inputs["laac_shallow"]).reshape(-1).astype(np.int64)
    sel_d = np.asarray(inputs["laac_deep"]).reshape(-1).astype(np.int64)
    Ws1 = np.asarray(inputs["Ws1"], dtype=np.float32)
    Ws2 = np.asarray(inputs["Ws2"], dtype=np.float32)
    Wd1 = np.asarray(inputs["Wd1"], dtype=np.float32)
    Wd2 = np.asarray(inputs["Wd2"], dtype=np.float32)
    bs1 = np.asarray(inputs["bs1"], dtype=np.float32)
    bs2 = np.asarray(inputs["bs2"], dtype=np.float32)
    bd1 = np.asarray(inputs["bd1"], dtype=np.float32)
    bd2 = np.asarray(inputs["bd2"], dtype=np.float32)

    add_bias = any(
        float(np.abs(b).max()) != 0.0 for b in (bs1, bs2, bd1, bd2)
    )

    in_maps = []
    for a in range(A):
        es, ed = int(sel_s[a]), int(sel_d[a])
        # bt-major packing: col = bt*(S//P)*BT + k*BT + b
        xp = np.ascontiguousarray(
            x[a]
            .reshape(NBT, BT, S // P, P)
            .transpose(3, 0, 2, 1)
            .reshape(P, -1)
            .astype(np_dt)
        )
        m = {
            "x": xp,
            "w1": _packw(Ws1[es], np_dt),
            "w2": _packw(Ws2[es], np_dt),
            "w3": _packw(Wd1[ed], np_dt),
            "w4": _packw(Wd2[ed], np_dt),
        }
        if add_bias:
            bias_cols = np.concatenate([bs1[es], bs2[es], bd1[ed], bd2[ed]])
            m["bias"] = np.ascontiguousarray(
                bias_cols.reshape(11, P).T, dtype=np.float32
            )
        in_maps.append(m)
    return add_bias, in_maps


def kernel(**inputs) -> np.ndarray:
    dt_name, warm, junk = _cfg()
    add_bias, in_maps = _prepare(inputs, dt_name)
    nc = _get_nc(dt_name, add_bias, warm, junk)
    res = run_bass_kernel_spmd(nc, in_maps, list(range(A)))
    out = np.stack(
        [np.asarray(res.results[a]["out"]).astype(np.float32).T for a in range(A)]
    )
    return np.ascontiguousarray(out)
